# revision 48
# baseline (speedup 1.0000x reference)
"""Trainium2 Bass kernel for nn_LocalInferenceModeling (cross-attention enhance).

Reference computation (per batch b):
    e = x1 @ x2^T                                  [L, L]
    a12 = softmax_j(e + m2[j]);  x1t = a12 @ x2    [L, H]
    a21 = softmax_i(e^T + m1[i]); x2t = a21 @ x1   [L, H]
    y1 = concat([x1, x1t, x1 - x1t, x1 * x1t], -1) [L, 4H]
    y2 = concat([x2, x2t, x2 - x2t, x2 * x2t], -1)

Sharding: batch dim B=32 split across 8 NeuronCores (4 batches/core), no
communication.  The device computes x1_tilde / x2_tilde; the host performs
the final (elementwise) enhance/concat on the exact fp32 inputs.

Device-side design:
  - e is computed once, in fp32 (f32r matmuls at full PE rate), in natural
    [i, j] layout.  A fused DVE tensor_tensor_reduce adds the pad masks
    (bf16-exact sentinel -29952 on padded j columns via a gpsimd partition
    broadcast of the mask row, plus per-partition sentinel on padded i) and
    emits the per-row max in the same pass.
  - p12 = exp(e - rowmax) runs on the Activation engine with a per-partition
    bias, emitting the softmax denominator z1 via accum_out for free.  The
    probabilities are normalized in fp16 (x0.5 DVE cost), transposed on the
    PE at 1 cycle/row (fp16), and contracted against fp16 x2 values.  The
    resulting psum is final (already normalized) and is DMA'd straight from
    PSUM to HBM in fp32.
  - p21 reuses e: column max via gpsimd partition reduces over valid-i tiles
    (the j-sentinel rides along and cancels exactly in fp32 when subtracted),
    exp with a true -1e30 per-partition bias for padded i, z2 via rank-1 PE
    matmuls, normalization folded into the psum->SBUF copies (spread across
    Pool/DVE/Act), fp16 output.
  - Sequence-length sparsity: softmax probabilities of fully-padded 128-row
    chunks are exactly zero, so the stage-2 contractions only run over the
    first C1/C2 chunks.  All 8 cores share one program, so the per-slot
    chunk counts are baked as the max over the cores' batches after a
    host-side assignment that groups batches of similar length; the program
    is rebuilt (and cached) per distinct slot signature.
"""

import sys

import numpy as np

sys.path.insert(0, "/opt/trn_rl_repo")

from contextlib import ExitStack

import concourse.bass as bass
import concourse.bacc as bacc
import concourse.bass_isa as bass_isa
import concourse.mybir as mybir
from concourse import masks
from concourse.bass_utils import run_bass_kernel_spmd
from concourse.tile import TileContext

B, L, H = 32, 512, 1024
NCORES = 8
BPC = B // NCORES  # batches per core
NT = L // 128  # 4 partition tiles per L
HT = H // 128  # 8 partition tiles per H

SENT = np.float32(29952.0)  # bf16-exact sentinel magnitude
NEG = np.float32(-1.0e30)

F32 = mybir.dt.float32
F32R = mybir.dt.float32r
FP16 = mybir.dt.float16

Exp = mybir.ActivationFunctionType.Exp
Copy = mybir.ActivationFunctionType.Copy
Add = mybir.AluOpType.add
Max = mybir.AluOpType.max

_NC_CACHE = {}


def build_nc(slots):
    """slots: tuple of BPC (C1, C2) pairs; C = valid 128-chunk count baked
    into slot k of every core."""
    nc = bacc.Bacc(None, target_bir_lowering=False)
    xt1 = nc.dram_tensor("xt1", [BPC, H, L], FP16, kind="ExternalInput")
    xt2 = nc.dram_tensor("xt2", [BPC, H, L], FP16, kind="ExternalInput")
    xb1 = nc.dram_tensor("xb1", [BPC, L, H], FP16, kind="ExternalInput")
    xb2 = nc.dram_tensor("xb2", [BPC, L, H], FP16, kind="ExternalInput")
    m2row = nc.dram_tensor("m2row", [BPC, L], FP16, kind="ExternalInput")
    m1cs = nc.dram_tensor("m1cs", [128, BPC * NT], F32, kind="ExternalInput")
    m1c = nc.dram_tensor("m1c", [128, BPC * NT], F32, kind="ExternalInput")
    o1 = nc.dram_tensor("o1", [BPC, L, H], FP16, kind="ExternalOutput")
    o2 = nc.dram_tensor("o2", [BPC, L, H], FP16, kind="ExternalOutput")

    qSP, qACT, qPL = nc.sync, nc.scalar, nc.gpsimd

    with TileContext(nc) as tc, ExitStack() as ctx:
        from concourse.tile import add_dep_helper

        const = ctx.enter_context(tc.tile_pool(name="const", bufs=1))
        ident = const.tile([128, 128], FP16)
        masks.make_identity(nc, ident[:])
        onesh = const.tile([128, 1], FP16)
        nc.vector.memset(onesh[:], 1.0)
        onesrow = const.tile([1, 128], FP16)
        nc.vector.memset(onesrow[:], 1.0)
        ones32col = const.tile([128, 1], F32)
        nc.vector.memset(ones32col[:], 1.0)
        ones32 = const.tile([1, 32], F32)
        nc.vector.memset(ones32[:], 1.0)

        xp = ctx.enter_context(tc.tile_pool(name="xp", bufs=2))
        esb = ctx.enter_context(tc.tile_pool(name="esb", bufs=6))
        pmp = ctx.enter_context(tc.tile_pool(name="pmp", bufs=6))
        cmp_ = ctx.enter_context(tc.tile_pool(name="cmp", bufs=2))
        pp = ctx.enter_context(tc.tile_pool(name="pp", bufs=2 * NT))
        ptp = ctx.enter_context(tc.tile_pool(name="ptp", bufs=2 * NT))
        p21p = ctx.enter_context(tc.tile_pool(name="p21p", bufs=2 * NT))
        st = ctx.enter_context(tc.tile_pool(name="st", bufs=6))
        o2p = ctx.enter_context(tc.tile_pool(name="o2p", bufs=8))
        mrp = ctx.enter_context(tc.tile_pool(name="mrp", bufs=1))
        psE = ctx.enter_context(tc.tile_pool(name="psE", bufs=2, space="PSUM"))
        psT = ctx.enter_context(tc.tile_pool(name="psT", bufs=2, space="PSUM"))
        psS = ctx.enter_context(tc.tile_pool(name="psS", bufs=3, space="PSUM"))
        psScr = ctx.enter_context(
            tc.tile_pool(name="psScr", bufs=1, space="PSUM"))
        scratch = psScr.tile([32, 32], F32, name="scratch", tag="scratch")

        gates = {"psE": [], "psT": [], "psS": []}
        touch_cnt = [0]

        def touch(ap):
            # Tiny PE matmul reading `ap` so the PE engine observes the
            # producer's sem tick; real matmuls then carry at most one sync
            # wait. Rotate over scratch columns so touches don't WAW-chain.
            p = min(ap.shape[0], 32)
            f = min(ap.shape[1], 32)
            if ap.dtype == F32R:
                ap = ap.bitcast(F32)
            oc = onesh if ap.dtype == FP16 else ones32col
            col = touch_cnt[0] % 32
            touch_cnt[0] += 1
            with tc.high_priority(offset=200):
                return nc.tensor.matmul(
                    scratch[0:f, col : col + 1], ap[0:p, 0:f], oc[0:p, 0:1],
                    start=True, stop=True)

        def gate(tag, bufs, first_inst):
            # Order the group's first PE write after the touch that observed
            # the release of the slot it reuses (bufs groups back).
            hist = gates[tag]
            k = len(hist)
            if k >= bufs and hist[k - bufs] is not None:
                add_dep_helper(first_inst.ins, hist[k - bufs].ins, sync=False,
                               reason="psum slot gate")
            hist.append(None)
            return k

        def set_gate(tag, k, tinst):
            gates[tag][k] = tinst

        touch(ident)
        nc.tensor.matmul(scratch[0:32, 0:1], ones32[0:1, :], ones32[0:1, 0:1],
                         start=True, stop=True)

        # ---- static mask loads ----
        m2r = mrp.tile([1, BPC * L], FP16, name="m2r", tag="m2r")
        m1cst = mrp.tile([128, BPC * NT], F32, name="m1cst", tag="m1cst")
        m1ct = mrp.tile([128, BPC * NT], F32, name="m1ct", tag="m1ct")
        m2rsrc = m2row.rearrange("b l -> (b l)")[None, :]
        qACT.dma_start(m1cst[:], m1cs[:, :])
        qACT.dma_start(m2r[:1, :L], m2rsrc[:, :L])

        def load_xt(b, q1=None, q2=None):
            xt1t = xp.tile([128, HT * L], FP16, name="xt1t", tag="xt1t")
            xt2t = xp.tile([128, HT * L], FP16, name="xt2t", tag="xt2t")
            # transposed e operands, in 2-chunk pieces (~0.8us each): the
            # first e matmuls start at first-piece, and short transfers
            # never monopolize an issuing engine
            q1 = q1 or [qSP] * 4
            q2 = q2 or [qSP] * 4
            for qs, t, src in ((q1, xt1t, xt1), (q2, xt2t, xt2)):
                for i in range(4):
                    qs[i].dma_start(
                        t[:, 2 * i * L : 2 * (i + 1) * L].rearrange(
                            "p (c l) -> p c l", c=2),
                        src[b, 2 * i * 128 : 2 * (i + 1) * 128].rearrange(
                            "(c p) l -> p c l", p=128))
            return xt1t, xt2t

        def load_xb(b, q1=None, q2=None):
            C1, C2 = slots[b]
            xb1t = xp.tile([128, NT * H], FP16, name="xb1t", tag="xb1t")
            xb2t = xp.tile([128, NT * H], FP16, name="xb2t", tag="xb2t")
            # natural stage-2 values: only the valid chunks are ever read;
            # per-chunk pieces (~0.8us)
            for q, t, src, C in ((q1 or qACT, xb1t, xb1, C1),
                                 (q2 or qACT, xb2t, xb2, C2)):
                for a in range(C):
                    q.dma_start(
                        t[:, a * H : (a + 1) * H],
                        src[b, a * 128 : (a + 1) * 128])
            return xb1t, xb2t

        def emit_head(b, xt1t, xt2t):
            """e matmuls + masks/rowmax + p12 (fp16) + colmax cm + p21.
            Returns what stage 2 needs."""
            C1, C2 = slots[b]
            touch(xt1t)
            touch(xt2t)
            m2row_b = m2r[0:1, L * b : L * (b + 1)]

            nm4 = st.tile([128, NT], F32, name="nm4", tag="nm4")
            z1 = st.tile([128, 2 * NT], F32, name="z1", tag="z1")
            e_sb = [esb.tile([128, L], F32, name="e_sb", tag="e_sb")
                    for _ in range(NT)]
            p12 = [pp.tile([128, L], FP16, name="p12", tag="p12")
                   for _ in range(NT)]
            pm = [pmp.tile([128, L], F32, name="pm", tag="pm")
                  for _ in range(C1)]
            W2 = C2 * 128
            for a in range(NT):
                pe = psE.tile([128, L], F32, name="psE", tag="psE")
                k = None
                for c in range(HT):
                    inst = nc.tensor.matmul(
                        pe[:],
                        xt1t[:, L * c + 128 * a : L * c + 128 * (a + 1)],
                        xt2t[:, L * c : L * (c + 1)],
                        start=(c == 0),
                        stop=False,
                    )
                    if c == 0:
                        k = gate("psE", 2, inst)
                # j-pad sentinel rank-1 (uniform -SENT on padded j columns)
                nc.tensor.matmul(pe[:], onesrow[0:1, :], m2row_b,
                                 start=False, stop=True)
                # negated rowmax straight off the psum (the m1 sentinel is
                # irrelevant for p12: a per-row shift cancels in softmax)
                nc.vector.reduce_max(nm4[:, a : a + 1], pe[:],
                                     axis=mybir.AxisListType.X, negate=True)
                # p12 = exp(e - rowmax) over valid-j chunks, read directly
                # from PSUM; z1 for free via accum
                nc.scalar.activation(
                    p12[a][:, :W2], pe[:, :W2], Exp,
                    bias=nm4[:, a : a + 1],
                    accum_out=z1[:, a : a + 1])
                # rz1 lands in the upper half of z1
                nc.vector.reciprocal(z1[:, NT + a : NT + a + 1],
                                     z1[:, a : a + 1])
                touch(p12[a])
                # psum drain for the p21 path (adds the i-pad sentinel so
                # the column max below excludes padded i) — off the critical
                # path, on Pool
                sc = (m1cst[:, NT * b + a : NT * b + a + 1]
                      if a < C1 else 0.0)
                nc.gpsimd.tensor_scalar_add(e_sb[a][:], pe[:], sc)
                set_gate("psE", k, touch(e_sb[a]))
                if a < C1:
                    nc.gpsimd.partition_all_reduce(
                        pm[a][:], e_sb[a][:], 128, bass_isa.ReduceOp.max)

            # column max over valid i (sentinels cancel on subtraction)
            if C1 == 1:
                cm = pm[0]
            else:
                cm = cmp_.tile([128, L], F32, name="cm", tag="cm")
                nc.vector.tensor_max(cm[:], pm[0][:], pm[1][:])
                for a in range(2, C1):
                    nc.vector.tensor_max(cm[:], cm[:], pm[a][:])

            # p21 = exp(e - colmax) with -1e30 bias on padded i
            p21 = [p21p.tile([128, L], FP16, name="p21", tag="p21")
                   for _ in range(C1)]
            for a in range(C1):
                nc.vector.tensor_sub(e_sb[a][:], e_sb[a][:], cm[:])
                nc.scalar.activation(
                    p21[a][:], e_sb[a][:], Exp,
                    bias=m1ct[:, NT * b + a : NT * b + a + 1])
                touch(p21[a])
            return p12, p21, z1

        cp_engs = [nc.gpsimd, nc.scalar, nc.vector]
        cp_i = [0]

        def norm_copy(dst, pt, rz):
            # priority boost: stage2(b) copies are emitted after head(b+1),
            # but they gate the PE psum rings — they must beat head(b+1)'s
            # chain ops on DVE/Pool when both are ready
            eng = cp_engs[cp_i[0] % 3]
            cp_i[0] += 1
            with tc.high_priority(offset=2000):
                if eng is nc.scalar:
                    eng.activation(dst, pt, Copy, scale=rz)
                else:
                    eng.tensor_scalar_mul(dst, pt, rz)

        def emit_stage2(b, head, xb1t, xb2t, last=False):
            C1, C2 = slots[b]
            p12, p21, z1 = head
            touch(xb1t)
            touch(xb2t)

            # stage-2 contraction groups alternate between the psS and psE
            # rings (psE is idle here: e(b+1) has already run), so the
            # norm-copy latency never backpressures the PE
            s2_i = [0]

            def s2_pool():
                s2_i[0] += 1
                return (psS, "psS", 3) if s2_i[0] % 2 else (psE, "psE", 2)

            ysq = []  # deferred output tiles: (a, ys1, ys)

            def x2t_group(a, n, ys):
                pool, tg, nb = s2_pool()
                pt = pool.tile([128, 512], F32, name="s2", tag=tg)
                k = None
                for ai in range(C1):
                    inst = nc.tensor.matmul(
                        pt[:],
                        p21[ai][:, 128 * a : 128 * (a + 1)],
                        xb1t[:, H * ai + 512 * n : H * ai + 512 * (n + 1)],
                        start=(ai == 0), stop=(ai == C1 - 1),
                    )
                    if ai == 0:
                        k = gate(tg, nb, inst)
                norm_copy(ys[:, 512 * n : 512 * (n + 1)], pt[:],
                          rz2[:, a : a + 1])
                set_gate(tg, k, touch(ys[:, 512 * n : 512 * (n + 1)]))

            # transposes of p12 chunks (fp16, exact), interleaved with x2t
            # groups so the psT drain (Pool copy) hides under PE work
            ys_x2 = [o2p.tile([128, H], FP16, name="ys", tag="ys")
                     for _ in range(NT)]
            rz2 = st.tile([128, NT], F32, name="rz2", tag="rz2")
            z2ps = None
            kz2 = None
            p12T = []
            x2q = [(a, n) for a in range(NT) for n in range(2)]
            xi = 0
            for c in range(C2 + 1):
                if c < C2:
                    tp = psT.tile([128, L], FP16, name="psT", tag="psT")
                    k = None
                    for a in range(NT):
                        inst = nc.tensor.matmul(
                            tp[:, 128 * a : 128 * (a + 1)],
                            p12[a][:, 128 * c : 128 * (c + 1)],
                            ident[:], is_transpose=True,
                            start=(a == 0), stop=False,
                        )
                        if a == 0:
                            k = gate("psT", 2, inst)
                    sb = ptp.tile([128, L], FP16, name="p12T", tag="p12T")
                    # alternate the psT drain between Pool and DVE so the
                    # copy latency never gates the transpose ring; boosted
                    # like norm_copy (gates the psT ring)
                    with tc.high_priority(offset=2000):
                        (nc.gpsimd if c % 2 == 0 else nc.vector).tensor_copy(
                            sb[:], tp[:])
                    set_gate("psT", k, touch(sb))
                    p12T.append(sb)
                if c == 0:
                    # z2 (rank-1 partition sums of p21) — p21 is long ready
                    # by now, so this never stalls the PE
                    z2ps = psS.tile([128, NT], F32, name="z2ps", tag="psS")
                    for ai in range(C1):
                        for t in range(NT):
                            inst = nc.tensor.matmul(
                                z2ps[:, t : t + 1],
                                p21[ai][:, 128 * t : 128 * (t + 1)],
                                onesh[:], start=(ai == 0 and t == 0),
                                stop=(ai == C1 - 1 and t == NT - 1))
                            if ai == 0 and t == 0:
                                kz2 = gate("psS", 3, inst)
                    nc.vector.reciprocal(rz2[:], z2ps[:])
                    set_gate("psS", kz2, touch(rz2))
                else:
                    # one x2t group between transposes
                    if xi < len(x2q):
                        a, n = x2q[xi]
                        xi += 1
                        x2t_group(a, n, ys_x2[a])

            for a in range(NT):
                ys1 = o2p.tile([128, H], FP16, name="ys1", tag="ys1")
                for n in range(2):
                    pool, tg, nb = s2_pool()
                    pt = pool.tile([128, 512], F32, name="s2", tag=tg)
                    k = None
                    for ci in range(C2):
                        inst = nc.tensor.matmul(
                            pt[:],
                            p12T[ci][:, 128 * a : 128 * (a + 1)],
                            xb2t[:, H * ci + 512 * n : H * ci + 512 * (n + 1)],
                            start=(ci == 0), stop=(ci == C2 - 1),
                        )
                        if ci == 0:
                            k = gate(tg, nb, inst)
                    norm_copy(ys1[:, 512 * n : 512 * (n + 1)], pt[:],
                              z1[:, NT + a : NT + a + 1])
                    set_gate(tg, k, touch(ys1[:, 512 * n : 512 * (n + 1)]))
                if xi < len(x2q):
                    aa, nn = x2q[xi]
                    xi += 1
                    x2t_group(aa, nn, ys_x2[aa])
                if xi < len(x2q):
                    aa, nn = x2q[xi]
                    xi += 1
                    x2t_group(aa, nn, ys_x2[aa])
                rows = slice(128 * a, 128 * (a + 1))
                if last and a == NT - 1:
                    # final tiles: split across all queues to cut the tail
                    bnd = (0, 342, 684, 1024)
                    for qq, q in enumerate((qSP, qACT, qPL)):
                        cs = slice(bnd[qq], bnd[qq + 1])
                        q.dma_start(o1[b, rows, cs], ys1[:, cs])
                        q.dma_start(o2[b, rows, cs], ys_x2[a][:, cs])
                else:
                    qSP.dma_start(o1[b, rows, :], ys1[:])
                    qSP.dma_start(o2[b, rows, :], ys_x2[a][:])

        # ---- software-pipelined batch loop ----
        # PE order: e(0) | e(1) | T/z2/s2(0) | e(2) | T/z2/s2(1) | ...
        # Prologue loads are hand-spread over the three queues; steady
        # state keeps xt on SP (pure DMA queue) and xb on Act.
        xts = {0: load_xt(0, [qSP] * 4, [qPL] * 4)}
        xts[1] = load_xt(1, [qSP, qSP, qSP, qPL],
                         [qACT, qACT, qPL, qPL])
        for bb in range(1, BPC):
            qACT.dma_start(m2r[:1, L * bb : L * (bb + 1)],
                           m2rsrc[:, L * bb : L * (bb + 1)])
        qACT.dma_start(m1ct[:], m1c[:, :])
        xbs = {0: load_xb(0, qSP, qPL)}
        heads = {}
        for b in range(BPC):
            heads[b] = emit_head(b, *xts.pop(b))
            if b + 1 < BPC:
                if b + 1 not in xts:
                    xts[b + 1] = load_xt(b + 1)
                xbs[b + 1] = load_xb(b + 1)
            if b > 0:
                emit_stage2(b - 1, heads.pop(b - 1), *xbs.pop(b - 1))
        emit_stage2(BPC - 1, heads.pop(BPC - 1), *xbs.pop(BPC - 1),
                    last=True)

    if not nc.is_finalized():
        nc.finalize()
    return nc


def _plan_slots(c1, c2):
    """Partition the B batches into BPC groups of NCORES, minimizing
    sum over groups of (max c1 + max c2).  Returns (slots, assign) where
    assign[core][slot] = original batch index."""
    order = np.argsort(-(c1 + c2), kind="stable")
    groups = [list(order[k * NCORES : (k + 1) * NCORES]) for k in range(BPC)]

    def gcost(g):
        return max(c1[i] for i in g) + max(c2[i] for i in g)

    # local search: swap members between groups while it helps
    improved = True
    it = 0
    while improved and it < 200:
        improved = False
        it += 1
        for ga in range(BPC):
            for gb in range(ga + 1, BPC):
                base = gcost(groups[ga]) + gcost(groups[gb])
                for ia in range(NCORES):
                    for ib in range(NCORES):
                        groups[ga][ia], groups[gb][ib] = (
                            groups[gb][ib], groups[ga][ia])
                        new = gcost(groups[ga]) + gcost(groups[gb])
                        if new < base:
                            base = new
                            improved = True
                        else:
                            groups[ga][ia], groups[gb][ib] = (
                                groups[gb][ib], groups[ga][ia])
    slots = tuple(
        (int(max(c1[i] for i in g)), int(max(c2[i] for i in g)))
        for g in groups)
    assign = [[groups[k][core] for k in range(BPC)]
              for core in range(NCORES)]
    return slots, assign


def kernel(x1_bar, seq_lengths1, x2_bar, seq_lengths2):
    x1_bar = np.ascontiguousarray(x1_bar, dtype=np.float32)
    x2_bar = np.ascontiguousarray(x2_bar, dtype=np.float32)
    sl1 = np.asarray(seq_lengths1).astype(np.int32)
    sl2 = np.asarray(seq_lengths2).astype(np.int32)

    c1 = np.clip((sl1 + 127) // 128, 1, NT).astype(np.int64)
    c2 = np.clip((sl2 + 127) // 128, 1, NT).astype(np.int64)
    slots, assign = _plan_slots(c1, c2)

    xt1f = np.ascontiguousarray(x1_bar.transpose(0, 2, 1)).astype(np.float16)
    xt2f = np.ascontiguousarray(x2_bar.transpose(0, 2, 1)).astype(np.float16)
    xb1f = x1_bar.astype(np.float16)
    xb2f = x2_bar.astype(np.float16)

    ar = np.arange(L, dtype=np.int32)
    pad1 = ar[None, :] >= sl1[:, None]  # [B, L] True on padded i
    pad2 = ar[None, :] >= sl2[:, None]
    m2rowf = np.where(pad2, -SENT, 0.0).astype(np.float16)

    def swz(m, val, idx):
        out = np.where(m[idx], val, 0.0).astype(np.float32)  # [BPC, L]
        return np.ascontiguousarray(
            out.reshape(BPC, NT, 128).transpose(2, 0, 1).reshape(
                128, BPC * NT))

    key = slots
    if key not in _NC_CACHE:
        _NC_CACHE.clear()
        _NC_CACHE[key] = build_nc(slots)
    nc = _NC_CACHE[key]

    in_maps = []
    for core in range(NCORES):
        idx = np.array(assign[core], dtype=np.int64)
        in_maps.append({
            "xt1": np.ascontiguousarray(xt1f[idx]),
            "xt2": np.ascontiguousarray(xt2f[idx]),
            "xb1": np.ascontiguousarray(xb1f[idx]),
            "xb2": np.ascontiguousarray(xb2f[idx]),
            "m2row": np.ascontiguousarray(m2rowf[idx]),
            "m1cs": swz(pad1, -SENT, idx),
            "m1c": swz(pad1, NEG, idx),
        })

    res = run_bass_kernel_spmd(nc, in_maps, core_ids=list(range(NCORES)))

    x1t = np.empty((B, L, H), dtype=np.float32)
    x2t = np.empty((B, L, H), dtype=np.float32)
    for core in range(NCORES):
        r = res.results[core]
        for k in range(BPC):
            bi = assign[core][k]
            x1t[bi] = r["o1"][k].astype(np.float32)
            x2t[bi] = r["o2"][k].astype(np.float32)

    y1 = np.empty((B, L, 4 * H), dtype=np.float32)
    y2 = np.empty((B, L, 4 * H), dtype=np.float32)
    y1[:, :, 0:H] = x1_bar
    y1[:, :, H : 2 * H] = x1t
    y1[:, :, 2 * H : 3 * H] = x1_bar - x1t
    y1[:, :, 3 * H :] = x1_bar * x1t
    y2[:, :, 0:H] = x2_bar
    y2[:, :, H : 2 * H] = x2t
    y2[:, :, 2 * H : 3 * H] = x2_bar - x2t
    y2[:, :, 3 * H :] = x2_bar * x2t
    return y1, y2
